# revision 2
# baseline (speedup 1.0000x reference)
"""MoE layer (8 experts, top-2) on 8 Trainium2 NeuronCores.

Strategy: 8-way tensor parallelism over the FFN hidden dim (D_FF).
-----------------------------------------------------------------
Every core runs the SAME program over ALL experts' routed tokens, but
only a 512-wide slice of the 4096-wide hidden layer:

  core c:  pre_c = x @ W1[:, c*512:(c+1)*512] + b1[slice]
           h_c   = gelu(pre_c)
           y_c   = h_c @ W2[c*512:(c+1)*512, :]        (partial sum)

The host sums the 8 partial y_c (exact: the FFN output is linear in the
hidden activations).  Compared to expert-per-core this removes the
routing-imbalance padding entirely: per-core work is exactly 1/8 of the
total token*FLOP volume regardless of how tokens route, while weight
DMA per core stays the same (1/8 of every expert's W1+W2 = one full
expert's worth).

Host (inside kernel(), cheap O(T*D) work):
  * gate: logits = x @ Wg, softmax, top-2, normalized combine weights
  * dispatch: gather each expert's tokens, pad to per-expert tile plan
  * combine: out[t] += w * (sum_c y_c + b2[e]) scatter-add

Device (>99% of FLOPs): all matmuls on TensorE in bf16 with fp32 PSUM,
weights resident in SBUF, D/F on partitions and tokens on the matmul
free dim so no transposes are needed anywhere.

Returns the full [B, S, D] float32 output.
"""

import os
import sys

for _p in ("/opt/trn_rl_repo",):
    if _p not in sys.path:
        sys.path.insert(0, _p)

import numpy as np
import ml_dtypes

import concourse.bass as bass
import concourse.mybir as mybir
import concourse.tile as tile
from concourse import bacc
from concourse.bass_utils import run_bass_kernel_spmd

D_MODEL = 1024
D_FF = 4096
NUM_EXPERTS = 8
TOP_K = 2
N_CORES = 8
P = 128          # SBUF partitions
DC = D_MODEL // P    # 8 chunks of the model dim
FSL = D_FF // N_CORES   # 512 hidden columns per core
FCC = FSL // P       # 4 hidden chunks per core

LAST_EXEC_NS = None


def _install_profile_hook():
    """Provide antenv.axon_hooks (NTFF profiling) if the image lacks it."""
    import types
    import contextlib
    import ctypes
    try:
        from antenv.axon_hooks import get_axon_ntff_profile_hook  # noqa: F401
        return
    except ImportError:
        pass
    so = "/opt/axon/libaxon_pjrt.so"
    if not os.path.exists(so):
        return
    lib = ctypes.CDLL(so)
    if not hasattr(lib, "axon_start_nrt_profile"):
        return
    lib.axon_start_nrt_profile.argtypes = [ctypes.POINTER(ctypes.c_int64),
                                           ctypes.c_size_t]
    lib.axon_start_nrt_profile.restype = ctypes.c_int64
    lib.axon_stop_nrt_profile.argtypes = [ctypes.c_char_p]
    lib.axon_stop_nrt_profile.restype = ctypes.c_int64

    @contextlib.contextmanager
    def _hook(output_dir, device_ids):
        import jax
        jax.devices()
        if device_ids:
            ids = (ctypes.c_int64 * len(device_ids))(*device_ids)
            rc = lib.axon_start_nrt_profile(ids, len(device_ids))
        else:
            rc = lib.axon_start_nrt_profile(None, 0)
        try:
            yield
        finally:
            if rc == 0:
                n = lib.axon_stop_nrt_profile(str(output_dir).encode())
                print(f"profile: {n} ntff file(s) -> {output_dir}",
                      file=sys.stderr)

    mod = types.ModuleType("antenv.axon_hooks")
    mod.get_axon_ntff_profile_hook = lambda: _hook
    mod.set_axon_ntff_profile_hook = lambda h: None
    sys.modules["antenv.axon_hooks"] = mod
    import antenv
    antenv.axon_hooks = mod
    import concourse.bass_utils as _bu
    _bu.upload_artifacts = lambda tmpdir: tmpdir


def _expert_tiles(cnt):
    """(tn, ntiles): even tile size <=512 covering cnt with minimal pad."""
    k = max(1, -(-cnt // 512))
    tn = -(-cnt // k)
    tn += tn % 2
    tn = max(tn, 2)
    return tn, k


def _build_program(plans):
    """SPMD program: the core's F-slice of every expert's FFN.

    plans: list of (tn, ntiles) per expert.  DRAM layouts (all match the
    SBUF destination exactly -> long contiguous runs per partition):
      W1_e [P, DC, FSL]   w1[p, dc, f] = W1[e][dc*128+p, c*FSL+f]
      W2_e [P, FCC, D]    w2[p, fj, d] = W2[e][c*FSL+fj*128+p, d]
      x_e  [ntiles, P, DC*tn]  x[i, p, dc*tn+t] = x[i*tn+t, dc*128+p]
      b1c  [P, E*FCC]     b1c[p, e*FCC+fj] = b1[e][c*FSL+fj*128+p]
    Output yT_e [D, ntiles*tn] f32 (partial: this core's F-slice share).
    """
    bf16 = mybir.dt.bfloat16
    f32 = mybir.dt.float32
    nc = bacc.Bacc("TRN2", target_bir_lowering=False, debug=False,
                   num_devices=N_CORES)

    w1_d = [nc.dram_tensor(f"W1_{e}", [P, DC, FSL], bf16,
                           kind="ExternalInput").ap()
            for e in range(NUM_EXPERTS)]
    w2_d = [nc.dram_tensor(f"W2_{e}", [P, FCC, D_MODEL], bf16,
                           kind="ExternalInput").ap()
            for e in range(NUM_EXPERTS)]
    x_d = [nc.dram_tensor(f"x_{e}", [plans[e][1], P, DC * plans[e][0]], bf16,
                          kind="ExternalInput").ap()
           for e in range(NUM_EXPERTS)]
    b1_d = nc.dram_tensor("b1c", [P, NUM_EXPERTS * FCC], f32,
                          kind="ExternalInput").ap()
    y_d = [nc.dram_tensor(f"yT_{e}", [D_MODEL, plans[e][0] * plans[e][1]],
                          f32, kind="ExternalOutput").ap()
           for e in range(NUM_EXPERTS)]

    # global tile list: (expert, tile-in-expert, tn)
    tiles = [(e, i, plans[e][0])
             for e in range(NUM_EXPERTS) for i in range(plans[e][1])]

    with tile.TileContext(nc) as tc:
        with (
            tc.tile_pool(name="wpool", bufs=1) as wpool,
            tc.tile_pool(name="xpool", bufs=3) as xpool,
            tc.tile_pool(name="hpool", bufs=2) as hpool,
            tc.tile_pool(name="ypool", bufs=4) as ypool,
            tc.tile_pool(name="ph", bufs=3, space="PSUM") as ph_pool,
            tc.tile_pool(name="py", bufs=3, space="PSUM") as py_pool,
        ):
            b1s = wpool.tile([P, NUM_EXPERTS * FCC], f32)
            nc.scalar.dma_start(b1s[:], b1_d)

            # x tiles ride the ACT ring; rotate 3 fixed-size flat buffers
            def x_dma(t):
                e, i, tn = tiles[t]
                xb = xpool.tile([P, DC * 512], bf16, tag="xb")
                nc.scalar.dma_start(xb[:, :DC * tn], x_d[e][i])
                return xb

            xbufs = [x_dma(0), x_dma(1), x_dma(2)]

            # weights ride the SP ring in consumption order
            w1s = [wpool.tile([P, DC, FSL], bf16, tag=f"w1_{e}",
                              name=f"w1_{e}") for e in range(NUM_EXPERTS)]
            w2s = [wpool.tile([P, FCC, D_MODEL], bf16, tag=f"w2_{e}",
                              name=f"w2_{e}") for e in range(NUM_EXPERTS)]
            for e in range(NUM_EXPERTS):
                nc.sync.dma_start(w1s[e][:], w1_d[e])
                nc.sync.dma_start(w2s[e][:], w2_d[e])

            # PE warm-up: dummy matmuls while the first weights load, so
            # HAM un-throttles before the first real matmul
            warm = wpool.tile([P, 256], bf16)
            nc.vector.memset(warm[:], 0.0)
            wps, _ = tc.tile([P, 256], f32, space="PSUM", name="warmps")
            for _ in range(40):
                nc.tensor.matmul(wps[:], warm[:, :P], warm[:], start=True,
                                 stop=True)

            for t, (e, ti, tn) in enumerate(tiles):
                xb = xbufs[t % 3]
                if t + 3 < len(tiles):
                    xbufs[t % 3] = None  # consumed below; refill after mm1
                # hT = gelu(W1.T @ x + b1), layout [F(part), fj, tokens]
                hT = hpool.tile([P, FCC, 512], bf16, tag="hT")
                for fj in range(FCC):
                    ph = ph_pool.tile([P, tn], f32, tag="ph")
                    for dc in range(DC):
                        nc.tensor.matmul(
                            ph[:],
                            w1s[e][:, dc, fj * P:(fj + 1) * P],
                            xb[:, dc * tn:(dc + 1) * tn],
                            start=(dc == 0),
                            stop=(dc == DC - 1),
                        )
                    nc.scalar.activation(
                        hT[:, fj, :tn], ph[:],
                        mybir.ActivationFunctionType.Gelu,
                        bias=b1s[:, e * FCC + fj:e * FCC + fj + 1],
                        scale=1.0,
                    )
                # prefetch the x tile 3 ahead (same buffer slot, now free
                # once this tile's mm1 consumed it)
                if t + 3 < len(tiles):
                    xbufs[t % 3] = x_dma(t + 3)

                # yT = W2.T @ hT (partial over this core's F-slice)
                for dc in range(DC):
                    py = py_pool.tile([P, tn], f32, tag="py")
                    for fj in range(FCC):
                        nc.tensor.matmul(
                            py[:],
                            w2s[e][:, fj, dc * P:(dc + 1) * P],
                            hT[:, fj, :tn],
                            start=(fj == 0),
                            stop=(fj == FCC - 1),
                        )
                    yt = ypool.tile([P, tn], f32, tag="yt")
                    nc.vector.tensor_copy(yt[:], py[:])
                    nc.scalar.dma_start(
                        y_d[e][dc * P:(dc + 1) * P, ti * tn:(ti + 1) * tn],
                        yt[:])

    nc.compile()
    return nc


def _route(x_flat, Wg):
    """Replicate the reference gate in float64: softmax, top-2, renorm."""
    logits = x_flat.astype(np.float64) @ Wg.astype(np.float64)
    logits -= logits.max(axis=-1, keepdims=True)
    p = np.exp(logits)
    p /= p.sum(axis=-1, keepdims=True)
    order = np.argsort(-p, axis=-1, kind="stable")[:, :TOP_K]   # [T, 2]
    rows = np.arange(p.shape[0])[:, None]
    tv = p[rows, order]                                          # [T, 2]
    tvn = tv / (tv.sum(axis=-1, keepdims=True) + 1e-8)
    return order, tvn


def kernel(x, Wg, W1, b1, W2, b2):
    global LAST_EXEC_NS
    x = np.asarray(x, dtype=np.float32)
    Wg = np.asarray(Wg, dtype=np.float32)
    W1 = np.asarray(W1, dtype=np.float32)
    b1 = np.asarray(b1, dtype=np.float32)
    W2 = np.asarray(W2, dtype=np.float32)
    b2 = np.asarray(b2, dtype=np.float32)

    B, S, D = x.shape
    x_flat = x.reshape(-1, D)
    T = x_flat.shape[0]

    order, tvn = _route(x_flat, Wg)

    idx = []
    wts = []
    for e in range(NUM_EXPERTS):
        sel = np.nonzero((order == e).any(axis=1))[0]
        idx.append(sel)
        wmat = np.where(order[sel] == e, tvn[sel], 0.0)
        wts.append(wmat.sum(axis=-1))                            # [cnt]

    plans = [_expert_tiles(len(s)) for s in idx]

    # a Bass program object must not be re-run after lowering (re-executing
    # a reused module corrupted the device) — build fresh every call; the
    # neuron compile cache keeps repeat builds fast
    nc = _build_program(plans)

    bf16 = ml_dtypes.bfloat16
    # x dispatch: shared across all cores (each core sees every token)
    x_e = []
    for e in range(NUM_EXPERTS):
        sel = idx[e]
        tn, k = plans[e]
        cap = tn * k
        xT = np.zeros((P, DC, cap), dtype=bf16)
        # [cnt, D] -> [cnt, DC, P] -> [P, DC, cnt]
        xT[:, :, :len(sel)] = x_flat[sel].reshape(-1, DC, P).transpose(2, 1, 0)
        # [P, DC, k, tn] -> [k, P, DC*tn]
        xT = np.ascontiguousarray(
            xT.reshape(P, DC, k, tn).transpose(2, 0, 1, 3)).reshape(
                k, P, DC * tn)
        x_e.append(xT)

    # per-core F-slices of the weights (one transpose pass for all cores)
    w1b = W1.astype(bf16)   # [E, D, F]
    w2b = W2.astype(bf16)   # [E, F, D]
    # [E, D, F] -> [E, DC, P, C, FSL] -> [C, E, P, DC, FSL]
    w1r = np.ascontiguousarray(
        w1b.reshape(NUM_EXPERTS, DC, P, N_CORES, FSL).transpose(3, 0, 2, 1, 4))
    # [E, F, D] -> [E, C, FCC, P, D] -> [C, E, P, FCC, D]
    w2r = np.ascontiguousarray(
        w2b.reshape(NUM_EXPERTS, N_CORES, FCC, P, D_MODEL).transpose(
            1, 0, 3, 2, 4))
    # [E, F] -> [E, C, FCC, P] -> [C, P, E*FCC]
    b1r = np.ascontiguousarray(
        b1.reshape(NUM_EXPERTS, N_CORES, FCC, P).transpose(1, 3, 0, 2)).reshape(
            N_CORES, P, NUM_EXPERTS * FCC)

    in_maps = []
    for c in range(N_CORES):
        m = {"b1c": b1r[c]}
        for e in range(NUM_EXPERTS):
            m[f"W1_{e}"] = w1r[c, e]
            m[f"W2_{e}"] = w2r[c, e]
            m[f"x_{e}"] = x_e[e]
        in_maps.append(m)

    trace = bool(os.environ.get("MOE_TRACE"))
    _install_profile_hook()   # also covers a harness-set BASS_TRACE=1
    try:
        res = run_bass_kernel_spmd(
            nc, in_maps, list(range(N_CORES)),
            trace=trace,
            tmpdir=os.environ.get("MOE_TRACE_DIR") or None,
        )
    except Exception:
        if not (trace or os.environ.get("BASS_TRACE")):
            raise
        # profiling path failed (e.g. no NTFF support) — run without it
        os.environ["BASS_NEVER_TRACE"] = "1"
        res = run_bass_kernel_spmd(nc, in_maps, list(range(N_CORES)))
    LAST_EXEC_NS = res.exec_time_ns

    out = np.zeros((T, D_MODEL), dtype=np.float64)
    for e in range(NUM_EXPERTS):
        sel = idx[e]
        acc = np.zeros(res.results[0][f"yT_{e}"].shape, dtype=np.float64)
        for c in range(N_CORES):
            acc += np.asarray(res.results[c][f"yT_{e}"])
        y = acc[:, :len(sel)].T
        out[sel] += wts[e][:, None] * (y + b2[e].astype(np.float64))

    return out.reshape(B, S, D_MODEL).astype(np.float32)
